# revision 21
# baseline (speedup 1.0000x reference)
"""AdaLN transformer block on 8 TRN2 NeuronCores.

Sharding: token-split. Core 2b+p owns batch b, query tokens
[1024p, 1024p+1024). Every core sees a uniform 2048-slot kv window whose
last 1024 slots are its own query tokens; for p=0 cores the first 1024
slots are zero padding that is killed in attention by a -30000 exp-bias
column (data, not control flow), so all 8 cores run one identical graph
with no collectives.

Layout: feature-major ("T" = transposed) everywhere; all host-side
transposes are free. Matmuls run in bf16 with fp32 PSUM accumulation.

Program order interleaves phases for engine overlap:
  A(kv chunks 0,1,2) -> B(Q0) -> A(chunk 3) -> B(Q1) -> C(T0) -> C(T1)
One shared PSUM pool (8 banks: mod 1 + qkv 2 + stat 1 + sc2g 2 + o2 2)
lets the Tile scheduler overlap phases freely.
"""

import sys

sys.path.insert(0, "/opt/trn_rl_repo")

import numpy as np
import ml_dtypes

import concourse.bass as bass
import concourse.mybir as mybir
import concourse.tile as tile
from concourse import bacc
from concourse.bass_utils import run_bass_kernel_spmd

BF16 = mybir.dt.bfloat16
F32 = mybir.dt.float32
AF = mybir.ActivationFunctionType
ALU = mybir.AluOpType

D = 512          # model dim
DT = 4           # d-tiles of 128
H = 8            # heads
DH = 64          # head dim
MLP = 2048
ADIM = 256
TKV = 2048       # kv window slots per core
TQ = 1024        # own query tokens per core
EPS = 1e-5
NEG = -30000.0
ISQ = DH ** -0.5

_cache: dict = {}


def _build():
    nc = bacc.Bacc(None, target_bir_lowering=False, debug=False)

    xT = nc.declare_dram_parameter("xT", [D, TKV], BF16, isOutput=False)
    aT = nc.declare_dram_parameter("aT", [ADIM, TKV], BF16, isOutput=False)
    kvb = nc.declare_dram_parameter("kvb", [128, 16], F32, isOutput=False)
    wqkvT = nc.declare_dram_parameter("wqkvT", [D, 3 * D], BF16, isOutput=False)
    wmodT = nc.declare_dram_parameter("wmodT", [ADIM, 6 * D], BF16, isOutput=False)
    woutT = nc.declare_dram_parameter("woutT", [D, D], BF16, isOutput=False)
    w1T = nc.declare_dram_parameter("w1T", [D, MLP], BF16, isOutput=False)
    w2T = nc.declare_dram_parameter("w2T", [MLP, D], BF16, isOutput=False)
    bcols = nc.declare_dram_parameter("bcols", [128, 48], F32, isOutput=False)
    out = nc.declare_dram_parameter("out", [D, TQ], F32, isOutput=True)

    with tile.TileContext(nc) as tc:
        with (
            tc.tile_pool(name="pers", bufs=1) as pers,
            tc.tile_pool(name="work", bufs=2) as work,
            tc.tile_pool(name="work1", bufs=1) as work1,
            tc.tile_pool(name="psum", bufs=1, space="PSUM") as psum,
        ):
            # ---------------- persistent SBUF ----------------
            t_wout = [pers.tile([128, D], BF16, tag=f"wout{i}", name=f"wout{i}") for i in range(DT)]
            t_xo = [pers.tile([128, TQ], BF16, tag=f"xo{i}", name=f"xo{i}") for i in range(DT)]
            t_bcols = pers.tile([128, 48], F32, tag="bcols", name="bcols")
            t_kvb = pers.tile([128, 16], F32, tag="kvb", name="kvb")
            t_ones = pers.tile([128, 1], BF16, tag="ones", name="ones")
            t_eps = pers.tile([1, 1], F32, tag="eps", name="eps")

            nc.sync.dma_start(t_bcols[:], bcols[:])
            nc.sync.dma_start(t_kvb[:], kvb[:])
            for i in range(DT):
                nc.sync.dma_start(t_xo[i][:], xT[i * 128:(i + 1) * 128, 1024:2048])
            for i in range(DT):
                nc.sync.dma_start(t_wout[i][:], woutT[i * 128:(i + 1) * 128, :])
            nc.gpsimd.memset(t_ones[:], 1.0)
            nc.gpsimd.memset(t_eps[:], EPS)

            # spill store for own-token modulation (read back in phase C)
            mod_dram = [nc.dram_tensor(f"mod_spill{g}", [128, DT, TQ], BF16)
                        for g in range(4)]

            # persistent activations
            t_k = [pers.tile([128, TKV], BF16, tag=f"kT{i}", name=f"kT{i}") for i in range(DT)]
            t_q = [pers.tile([128, TQ], BF16, tag=f"qT{i}", name=f"qT{i}") for i in range(DT)]
            # v_aug: per kv tile of 128 slots: 8 heads x (64 v + 1 ones)
            t_v = [pers.tile([128, H, DH + 1], BF16, tag=f"v{i}", name=f"v{i}") for i in range(16)]
            t_o = [pers.tile([128, TQ], BF16, tag=f"oT{i}", name=f"oT{i}") for i in range(DT)]
            t_x1 = [pers.tile([128, TQ], BF16, tag=f"x1T{i}", name=f"x1T{i}") for i in range(DT)]
            for i in range(16):
                nc.gpsimd.memset(t_v[i][:, :, DH:DH + 1], 1.0)

            # phase-A weights (space reused by w1/w2 later)
            poolWA = tc.tile_pool(name="wA", bufs=1)
            wA = poolWA.__enter__()
            t_xk = [wA.tile([128, TQ], BF16, tag=f"xk{i}", name=f"xk{i}") for i in range(DT)]
            for i in range(DT):
                nc.sync.dma_start(t_xk[i][:], xT[i * 128:(i + 1) * 128, 0:1024])
            t_wqkv = [wA.tile([128, 3 * D], BF16, tag=f"wqkv{i}", name=f"wqkv{i}") for i in range(DT)]
            t_wmod = [wA.tile([128, 6 * D], BF16, tag=f"wmod{i}", name=f"wmod{i}") for i in range(2)]
            for i in range(DT):
                nc.sync.dma_start(t_wqkv[i][:], wqkvT[i * 128:(i + 1) * 128, :])
            for i in range(2):
                nc.sync.dma_start(t_wmod[i][:], wmodT[i * 128:(i + 1) * 128, :])

            def layernorm_bcast(x_tiles, cols):
                """LN stats over the feature (partition) axis for one
                512-token chunk. Returns (R, M) SBUF bf16 [128,512]
                broadcast tiles: xn = x*R - M."""
                ps_st = psum.tile([128, 512], F32, tag="gen", name="gen", bufs=2)
                xsq = [None] * DT
                for dt in range(DT):
                    xsq[dt] = work1.tile([128, 512], BF16, tag=f"xsq_{dt}", name=f"xsq_{dt}")
                    nc.scalar.square(xsq[dt][:], x_tiles[dt][:, cols])
                for dt in range(DT):
                    nc.tensor.matmul(ps_st[0:1, :], t_ones[:], x_tiles[dt][:, cols],
                                     start=(dt == 0), stop=(dt == DT - 1))
                for dt in range(DT):
                    nc.tensor.matmul(ps_st[64:65, :], t_ones[:], xsq[dt][:],
                                     start=(dt == 0), stop=(dt == DT - 1))
                mu = work1.tile([1, 512], F32, tag="mu", name="mu", bufs=2)
                msq = work1.tile([1, 512], F32, tag="msq", name="msq", bufs=2)
                nc.vector.tensor_scalar_mul(mu[:], ps_st[0:1, :], 1.0 / D)
                nc.vector.tensor_scalar_mul(msq[:], ps_st[64:65, :], 1.0 / D)
                mu2 = work1.tile([1, 512], F32, tag="mu2", name="mu2", bufs=2)
                nc.scalar.square(mu2[:], mu[:])
                var = work1.tile([1, 512], F32, tag="var", name="var", bufs=2)
                nc.vector.tensor_tensor(var[:], msq[:], mu2[:], ALU.subtract)
                lnv = work1.tile([1, 512], F32, tag="lnv", name="lnv", bufs=2)
                nc.scalar.activation(lnv[:], var[:], AF.Ln, bias=t_eps[:, 0:1])
                rstd = work1.tile([1, 512], BF16, tag="rstd", name="rstd", bufs=2)
                nc.scalar.activation(rstd[:], lnv[:], AF.Exp, scale=-0.5)
                mrs = work1.tile([1, 512], BF16, tag="mrs", name="mrs", bufs=2)
                nc.vector.tensor_tensor(mrs[:], mu[:], rstd[:], ALU.mult)
                R = work1.tile([128, 512], BF16, tag="Rbc", name="Rbc", bufs=2)
                M = work1.tile([128, 512], BF16, tag="Mbc", name="Mbc", bufs=2)
                nc.gpsimd.partition_broadcast(R[:], rstd[:])
                nc.gpsimd.partition_broadcast(M[:], mrs[:])
                return R, M

            # ============ Phase A body (one 512-slot kv chunk) ============
            def phaseA(c4, act_frac=2):
                cols = slice(c4 * 512, (c4 + 1) * 512)
                own = c4 >= 2
                xt = t_xo if own else t_xk
                xcols = slice((c4 % 2) * 512, (c4 % 2 + 1) * 512)
                R, M = layernorm_bcast(xt, xcols)
                silu = [work.tile([128, 512], BF16, tag="silu", name="silu")
                        for _ in range(2)]
                for i in range(2):
                    a_chunk = work.tile([128, 512], BF16, tag="achunk", name="achunk")
                    nc.sync.dma_start(a_chunk[:], aT[i * 128:(i + 1) * 128, cols])
                    # silu(x) = x / (1 + exp(-x)); exp keeps ACT in the
                    # natural_log_exp table set shared with attention/LN
                    en = work.tile([128, 512], BF16, tag="en", name="en", bufs=1)
                    nc.scalar.activation(en[:], a_chunk[:], AF.Exp, scale=-1.0)
                    ep = work.tile([128, 512], BF16, tag="ep", name="ep", bufs=1)
                    nc.vector.tensor_scalar_add(ep[:], en[:], 1.0)
                    er = work.tile([128, 512], BF16, tag="er", name="er", bufs=1)
                    with nc.allow_low_precision(reason="bf16 silu denom"):
                        nc.vector.reciprocal(er[:], ep[:])
                    nc.vector.tensor_tensor(silu[i][:], a_chunk[:], er[:], ALU.mult)

                jlist = range(24) if own else range(8)
                sh1 = [None] * DT
                sc1p = [None] * DT
                modsp = None
                for j in jlist:
                    jc = slice(j * 128, (j + 1) * 128)
                    ps = psum.tile([128, 512], F32, tag="gen", name="gen", bufs=2)
                    for i in range(2):
                        nc.tensor.matmul(ps[:], t_wmod[i][:, jc], silu[i][:],
                                         start=(i == 0), stop=(i == 1))
                    grp, dt_i = divmod(j, 4)
                    bias_col = t_bcols[:, 24 + j:25 + j]
                    if j < 4:
                        dst = work1.tile([128, 512], BF16, tag=f"sh1_{dt_i}", name=f"sh1_{dt_i}")
                        sh1[dt_i] = dst
                        dst_ap = dst[:]
                    elif j < 8:
                        dst = work1.tile([128, 512], BF16, tag=f"sc1_{dt_i}", name=f"sc1_{dt_i}")
                        sc1p[dt_i] = dst
                        dst_ap = dst[:]
                    else:
                        if dt_i == 0:
                            modsp = work.tile([128, DT, 512], BF16, tag="modsp", name="modsp", bufs=1)
                        dst_ap = modsp[:, dt_i, :]
                    if act_frac and j % act_frac == 0:
                        nc.scalar.activation(dst_ap, ps[:], AF.Identity,
                                             bias=bias_col)
                    else:
                        nc.vector.tensor_scalar_add(dst_ap, ps[:], bias_col)
                    if j >= 8 and dt_i == DT - 1:
                        nc.sync.dma_start(
                            mod_dram[grp - 2][:, :, (c4 - 2) * 512:(c4 - 1) * 512],
                            modsp[:])

                h1 = [None] * DT
                for dt in range(DT):
                    t1 = work1.tile([128, 512], BF16, tag="t1", name="t1")
                    nc.vector.tensor_tensor(t1[:], xt[dt][:, xcols], R[:], ALU.mult)
                    xn = work1.tile([128, 512], BF16, tag="xn", name="xn")
                    nc.vector.tensor_tensor(xn[:], t1[:], M[:], ALU.subtract)
                    t2 = work1.tile([128, 512], BF16, tag="t2", name="t2")
                    nc.vector.tensor_tensor(t2[:], xn[:], sc1p[dt][:], ALU.mult)
                    h1[dt] = work.tile([128, 512], BF16, tag=f"h1_{dt}", name=f"h1_{dt}")
                    nc.vector.tensor_tensor(h1[dt][:], t2[:], sh1[dt][:], ALU.add)

                # k projection (wqkv cols 512:1024)
                for j in range(4):
                    jc = slice(512 + j * 128, 512 + (j + 1) * 128)
                    ps = psum.tile([128, 512], F32, tag="gen", name="gen", bufs=2)
                    for dt in range(DT):
                        nc.tensor.matmul(ps[:], t_wqkv[dt][:, jc], h1[dt][:],
                                         start=(dt == 0), stop=(dt == DT - 1))
                    nc.vector.tensor_copy(t_k[j][:, cols], ps[:])
                # v projection (wqkv cols 1024:1536), token-major
                for tt in range(4):
                    trows = slice(tt * 128, (tt + 1) * 128)
                    ps = psum.tile([128, 512], F32, tag="gen", name="gen", bufs=2)
                    for dt in range(DT):
                        nc.tensor.matmul(ps[:], h1[dt][:, trows],
                                         t_wqkv[dt][:, 1024:1536],
                                         start=(dt == 0), stop=(dt == DT - 1))
                    vt = t_v[c4 * 4 + tt]
                    nc.vector.tensor_copy(
                        vt[:, :, 0:DH],
                        ps[:].rearrange("p (h d) -> p h d", d=DH))
                # q projection (own chunks only; wqkv cols 0:512)
                if own:
                    qcols = slice((c4 - 2) * 512, (c4 - 1) * 512)
                    for j in range(4):
                        jc = slice(j * 128, (j + 1) * 128)
                        ps = psum.tile([128, 512], F32, tag="gen", name="gen", bufs=2)
                        for dt in range(DT):
                            nc.tensor.matmul(ps[:], t_wqkv[dt][:, jc], h1[dt][:],
                                             start=(dt == 0), stop=(dt == DT - 1))
                        nc.vector.tensor_copy(t_q[j][:, qcols], ps[:])

            # ============ Phase B body (one 512-token q chunk) ============
            def phaseB(Q):
                ncv = 12 if Q == 0 else 16
                qcols = slice(Q * 512, (Q + 1) * 512)
                band = range(8, 12) if Q == 0 else range(12, 16)
                for hg in range(4):          # head pairs (2hg, 2hg+1)
                    ktile = t_k[hg]
                    qtile = t_q[hg]
                    ps_o = [psum.tile([DH + 1, 512], F32, tag="o2", name="o2", bufs=2)
                            for _ in range(2)]
                    for c in range(ncv):
                        kc = slice(c * 128, (c + 1) * 128)
                        ps_s = psum.tile([128, 1024], F32, tag="sc2g", name="sc2g", bufs=2)
                        for hi in range(2):
                            nc.tensor.matmul(ps_s[:, hi * 512:(hi + 1) * 512],
                                             ktile[hi * 64:hi * 64 + 64, kc],
                                             qtile[hi * 64:hi * 64 + 64, qcols],
                                             start=True, stop=True,
                                             tile_position=(hi * 64, 0))
                        E = work.tile([128, 2, 512], BF16, tag="E", name="E", bufs=2)
                        nc.scalar.activation(
                            E[:].rearrange("p a b -> p (a b)"),
                            ps_s[:], AF.Exp,
                            bias=t_kvb[:, c:c + 1], scale=ISQ)
                        if c in band:
                            theta = 1024 + 512 * Q - 128 * c
                            Em = work.tile([128, 2, 512], BF16, tag="Em", name="Em", bufs=2)
                            nc.gpsimd.affine_select(
                                Em[:], E[:], pattern=[[0, 2], [1, 512]],
                                compare_op=ALU.is_ge, fill=0.0,
                                base=theta, channel_multiplier=-1)
                            src = Em
                        else:
                            src = E
                        for hi in range(2):
                            nc.tensor.matmul(ps_o[hi][:],
                                             t_v[c][:, 2 * hg + hi, :],
                                             src[:, hi, :],
                                             start=(c == 0), stop=(c == ncv - 1))
                    for hi in range(2):
                        h = 2 * hg + hi
                        hrows = slice(hi * 64, hi * 64 + 64)
                        recip = work.tile([1, 512], BF16, tag="recip", name="recip")
                        with nc.allow_low_precision(reason="bf16 softmax denom"):
                            nc.vector.reciprocal(recip[:], ps_o[hi][DH:DH + 1, :])
                        rbc = work1.tile([64, 512], BF16, tag="rbc", name="rbc")
                        nc.gpsimd.partition_broadcast(rbc[:], recip[:])
                        nc.vector.tensor_tensor(t_o[hg][hrows, qcols],
                                                ps_o[hi][0:DH, :], rbc[:], ALU.mult)

            # ============ Phase C sub-phases ============
            def read_mod(g, T):
                rb = work1.tile([128, DT, 512], BF16, tag=f"rb{g}", name=f"rb{g}")
                nc.sync.dma_start(rb[:], mod_dram[g][:, :, T * 512:(T + 1) * 512])
                return rb

            def projC(T):
                tcols = slice(T * 512, (T + 1) * 512)
                xcols = slice(T * 512, (T + 1) * 512)
                gate1 = read_mod(0, T)
                for m in range(4):
                    mc = slice(m * 128, (m + 1) * 128)
                    ps = psum.tile([128, 512], F32, tag="gen", name="gen", bufs=2)
                    for it in range(DT):
                        nc.tensor.matmul(ps[:], t_wout[it][:, mc], t_o[it][:, tcols],
                                         start=(it == 0), stop=(it == DT - 1))
                    pb = work1.tile([128, 512], BF16, tag="pb", name="pb")
                    nc.vector.tensor_scalar_add(pb[:], ps[:], t_bcols[:, m:m + 1])
                    pg = work1.tile([128, 512], BF16, tag="pg", name="pg")
                    nc.vector.tensor_tensor(pg[:], pb[:], gate1[:, m, :], ALU.mult)
                    nc.vector.tensor_tensor(t_x1[m][:, tcols], pg[:],
                                            t_xo[m][:, xcols], ALU.add)

            def lnC(T):
                tcols = slice(T * 512, (T + 1) * 512)
                shift2 = read_mod(1, T)
                scale2p = read_mod(2, T)
                R, M = layernorm_bcast(t_x1, tcols)
                h2 = [None] * DT
                for dt in range(DT):
                    t1 = work1.tile([128, 512], BF16, tag="t1", name="t1")
                    nc.vector.tensor_tensor(t1[:], t_x1[dt][:, tcols], R[:], ALU.mult)
                    xn = work1.tile([128, 512], BF16, tag="xn", name="xn")
                    nc.vector.tensor_tensor(xn[:], t1[:], M[:], ALU.subtract)
                    t2 = work1.tile([128, 512], BF16, tag="t2", name="t2")
                    nc.vector.tensor_tensor(t2[:], xn[:], scale2p[:, dt, :], ALU.mult)
                    h2[dt] = work.tile([128, 512], BF16, tag=f"h2_{dt}",
                                       name=f"h2_{dt}", bufs=2)
                    nc.vector.tensor_tensor(h2[dt][:], t2[:], shift2[:, dt, :], ALU.add)
                return h2

            def mlpC(T, h2, t_w1, t_w2):
                tcols = slice(T * 512, (T + 1) * 512)
                gate2 = read_mod(3, T)
                ps2pair = psum.tile([128, 1024], F32, tag="sc2g", name="sc2g", bufs=2)
                ps2 = [ps2pair[:, 0:512], ps2pair[:, 512:1024],
                       psum.tile([128, 512], F32, tag="o2", name="o2", bufs=2)[:],
                       psum.tile([128, 512], F32, tag="o2", name="o2", bufs=2)[:]]
                for k16 in range(16):
                    kc = slice(k16 * 128, (k16 + 1) * 128)
                    ps1 = psum.tile([128, 512], F32, tag="gen", name="gen", bufs=2)
                    for dt in range(DT):
                        nc.tensor.matmul(ps1[:], t_w1[dt][:, kc], h2[dt][:],
                                         start=(dt == 0), stop=(dt == DT - 1))
                    mid = work.tile([128, 512], BF16, tag="mid", name="mid", bufs=3)
                    nc.scalar.activation(mid[:], ps1[:], AF.Gelu,
                                         bias=t_bcols[:, 4 + k16:5 + k16])
                    for m in range(4):
                        mc = slice(m * 128, (m + 1) * 128)
                        nc.tensor.matmul(ps2[m], t_w2[k16][:, mc], mid[:],
                                         start=(k16 == 0), stop=(k16 == 15))
                for m in range(4):
                    ob = work1.tile([128, 512], BF16, tag="pb", name="pb")
                    nc.vector.tensor_scalar_add(ob[:], ps2[m], t_bcols[:, 20 + m:21 + m])
                    og = work1.tile([128, 512], BF16, tag="pg", name="pg")
                    nc.vector.tensor_tensor(og[:], ob[:], gate2[:, m, :], ALU.mult)
                    ofin = work1.tile([128, 512], F32, tag="ofin", name="ofin")
                    nc.vector.tensor_tensor(ofin[:], og[:], t_x1[m][:, tcols], ALU.add)
                    nc.sync.dma_start(out[m * 128:(m + 1) * 128, tcols], ofin[:])

            # ---------------- program order ----------------
            phaseA(0)
            phaseA(1)
            phaseA(2)
            phaseB(0)
            phaseA(3, act_frac=0)
            poolWA.__exit__(None, None, None)
            poolWC = tc.tile_pool(name="wC", bufs=1)
            wC = poolWC.__enter__()
            t_w1 = [wC.tile([128, MLP], BF16, tag=f"w1{i}", name=f"w1{i}") for i in range(DT)]
            t_w2 = [wC.tile([128, D], BF16, tag=f"w2{i}", name=f"w2{i}") for i in range(16)]
            for i in range(DT):
                nc.sync.dma_start(t_w1[i][:], w1T[i * 128:(i + 1) * 128, :])
            for i in range(16):
                nc.sync.dma_start(t_w2[i][:], w2T[i * 128:(i + 1) * 128, :])
            phaseB(1)
            projC(0)
            projC(1)
            h2_0 = lnC(0)
            h2_1 = lnC(1)
            mlpC(0, h2_0, t_w1, t_w2)
            mlpC(1, h2_1, t_w1, t_w2)
            poolWC.__exit__(None, None, None)

    nc.compile()
    return nc


def _get_nc():
    if "nc" not in _cache:
        _cache["nc"] = _build()
    return _cache["nc"]


def _shard(x, action_emb, Wqkv, Wout, bout, W1, b1, W2, b2, Wmod, bmod):
    bf = ml_dtypes.bfloat16
    wqkvT = np.ascontiguousarray(Wqkv.T).astype(bf)
    wmodT = np.ascontiguousarray(Wmod.T).astype(bf)
    woutT = np.ascontiguousarray(Wout.T).astype(bf)
    w1T = np.ascontiguousarray(W1.T).astype(bf)
    w2T = np.ascontiguousarray(W2.T).astype(bf)
    bcolsv = np.zeros((128, 48), np.float32)
    bcolsv[:, 0:4] = bout.reshape(4, 128).T
    bcolsv[:, 4:20] = b1.reshape(16, 128).T
    bcolsv[:, 20:24] = b2.reshape(4, 128).T
    bmod_cols = np.ascontiguousarray(bmod.reshape(24, 128).T)
    bmod_cols[:, 4:8] += 1.0    # scale1 -> 1 + scale1
    bmod_cols[:, 16:20] += 1.0  # scale2 -> 1 + scale2
    bcolsv[:, 24:48] = bmod_cols

    in_maps = []
    for b in range(4):
        for p in range(2):
            if p == 0:
                xw = np.zeros((TKV, D), np.float32)
                xw[1024:] = x[b, 0:1024]
                aw = np.zeros((TKV, ADIM), np.float32)
                aw[1024:] = action_emb[b, 0:1024]
                kvbv = np.zeros((128, 16), np.float32)
                kvbv[:, 0:8] = NEG
            else:
                xw = x[b]
                aw = action_emb[b]
                kvbv = np.zeros((128, 16), np.float32)
            in_maps.append({
                "xT": np.ascontiguousarray(xw.T).astype(bf),
                "aT": np.ascontiguousarray(aw.T).astype(bf),
                "kvb": kvbv,
                "wqkvT": wqkvT,
                "wmodT": wmodT,
                "woutT": woutT,
                "w1T": w1T,
                "w2T": w2T,
                "bcols": bcolsv,
            })
    return in_maps


def kernel(x, action_emb, causal_mask, Wqkv, Wout, bout, W1, b1, W2, b2,
           Wmod, bmod):
    in_maps = _shard(np.asarray(x, np.float32), np.asarray(action_emb, np.float32),
                     np.asarray(Wqkv, np.float32), np.asarray(Wout, np.float32),
                     np.asarray(bout, np.float32), np.asarray(W1, np.float32),
                     np.asarray(b1, np.float32), np.asarray(W2, np.float32),
                     np.asarray(b2, np.float32), np.asarray(Wmod, np.float32),
                     np.asarray(bmod, np.float32))
    nc = _get_nc()
    res = run_bass_kernel_spmd(nc, in_maps, core_ids=list(range(8)))
    full = np.empty((4, 2048, D), np.float32)
    for b in range(4):
        for p in range(2):
            full[b, p * 1024:(p + 1) * 1024, :] = res.results[2 * b + p]["out"].T
    return full


# revision 23
# speedup vs baseline: 1.1641x; 1.1641x over previous
"""AdaLN transformer block on 8 TRN2 NeuronCores.

Sharding: token-split. Core 2b+p owns batch b, query tokens
[1024p, 1024p+1024). Every core sees a uniform 2048-slot kv window whose
last 1024 slots are its own query tokens; for p=0 cores the first 1024
slots are zero padding that is killed in attention by a -30000 exp-bias
column (data, not control flow), so all 8 cores run one identical graph
with no collectives.

Layout: feature-major ("T" = transposed) everywhere; all host-side
transposes are free. Matmuls run in bf16 with fp32 PSUM accumulation.

Program order interleaves phases for engine overlap:
  A(kv chunks 0,1,2) -> B(Q0) -> A(chunk 3) -> B(Q1) -> C(T0) -> C(T1)
One shared PSUM pool (8 banks: mod 1 + qkv 2 + stat 1 + sc2g 2 + o2 2)
lets the Tile scheduler overlap phases freely.
"""

import sys

sys.path.insert(0, "/opt/trn_rl_repo")

import numpy as np
import ml_dtypes

import concourse.bass as bass
import concourse.mybir as mybir
import concourse.tile as tile
from concourse import bacc
from concourse.bass_utils import run_bass_kernel_spmd

BF16 = mybir.dt.bfloat16
F32 = mybir.dt.float32
FP8 = mybir.dt.float8e4
DR = mybir.MatmulPerfMode.DoubleRow
AF = mybir.ActivationFunctionType
ALU = mybir.AluOpType

D = 512          # model dim
DT = 4           # d-tiles of 128
H = 8            # heads
DH = 64          # head dim
MLP = 2048
ADIM = 256
TKV = 2048       # kv window slots per core
TQ = 1024        # own query tokens per core
EPS = 1e-5
NEG = -30000.0
ISQ = DH ** -0.5

_cache: dict = {}


def _build():
    nc = bacc.Bacc(None, target_bir_lowering=False, debug=False)

    xT = nc.declare_dram_parameter("xT", [D, TKV], BF16, isOutput=False)
    aT = nc.declare_dram_parameter("aT", [ADIM, TKV], BF16, isOutput=False)
    kvb = nc.declare_dram_parameter("kvb", [128, 16], F32, isOutput=False)
    wqkvT = nc.declare_dram_parameter("wqkvT", [D, 3 * D], BF16, isOutput=False)
    wmodT = nc.declare_dram_parameter("wmodT", [ADIM, 6 * D], BF16, isOutput=False)
    woutT = nc.declare_dram_parameter("woutT", [D, D], BF16, isOutput=False)
    w1T = nc.declare_dram_parameter("w1T", [128, DT * MLP], FP8, isOutput=False)
    w2T = nc.declare_dram_parameter("w2T", [128, 16 * D], FP8, isOutput=False)
    bcols = nc.declare_dram_parameter("bcols", [128, 48], F32, isOutput=False)
    out = nc.declare_dram_parameter("out", [D, TQ], F32, isOutput=True)

    with tile.TileContext(nc) as tc:
        with (
            tc.tile_pool(name="pers", bufs=1) as pers,
            tc.tile_pool(name="work", bufs=2) as work,
            tc.tile_pool(name="work1", bufs=1) as work1,
            tc.tile_pool(name="psum", bufs=1, space="PSUM") as psum,
        ):
            # ---------------- persistent SBUF ----------------
            t_wout = [pers.tile([128, D], BF16, tag=f"wout{i}", name=f"wout{i}") for i in range(DT)]
            t_xo = [pers.tile([128, TQ], BF16, tag=f"xo{i}", name=f"xo{i}") for i in range(DT)]
            t_bcols = pers.tile([128, 48], F32, tag="bcols", name="bcols")
            t_kvb = pers.tile([128, 16], F32, tag="kvb", name="kvb")
            t_ones = pers.tile([128, 1], BF16, tag="ones", name="ones")
            t_eps = pers.tile([1, 1], F32, tag="eps", name="eps")

            nc.sync.dma_start(t_bcols[:], bcols[:])
            nc.sync.dma_start(t_kvb[:], kvb[:])
            for i in range(DT):
                nc.sync.dma_start(t_xo[i][:], xT[i * 128:(i + 1) * 128, 1024:2048])
            for i in range(DT):
                nc.sync.dma_start(t_wout[i][:], woutT[i * 128:(i + 1) * 128, :])
            nc.gpsimd.memset(t_ones[:], 1.0)
            nc.gpsimd.memset(t_eps[:], EPS)

            # spill store for own-token modulation (read back in phase C)
            mod_dram = [nc.dram_tensor(f"mod_spill{g}", [128, DT, TQ], BF16)
                        for g in range(4)]

            # persistent activations
            t_k = [pers.tile([128, TKV], BF16, tag=f"kT{i}", name=f"kT{i}") for i in range(DT)]
            t_q = [pers.tile([128, TQ], BF16, tag=f"qT{i}", name=f"qT{i}") for i in range(DT)]
            # v_aug: per kv tile of 128 slots: 8 heads x (64 v + 1 ones)
            t_v = [pers.tile([128, H, DH + 1], BF16, tag=f"v{i}", name=f"v{i}") for i in range(16)]
            t_o = [pers.tile([128, TQ], BF16, tag=f"oT{i}", name=f"oT{i}") for i in range(DT)]
            t_x1 = [pers.tile([128, TQ], BF16, tag=f"x1T{i}", name=f"x1T{i}") for i in range(DT)]
            for i in range(16):
                nc.gpsimd.memset(t_v[i][:, :, DH:DH + 1], 1.0)

            # phase-A weights (space reused by w1/w2 later)
            poolWA = tc.tile_pool(name="wA", bufs=1)
            wA = poolWA.__enter__()
            t_xk = [wA.tile([128, TQ], BF16, tag=f"xk{i}", name=f"xk{i}") for i in range(DT)]
            for i in range(DT):
                nc.sync.dma_start(t_xk[i][:], xT[i * 128:(i + 1) * 128, 0:1024])
            t_wqkv = [wA.tile([128, 3 * D], BF16, tag=f"wqkv{i}", name=f"wqkv{i}") for i in range(DT)]
            t_wmod = [wA.tile([128, 6 * D], BF16, tag=f"wmod{i}", name=f"wmod{i}") for i in range(2)]
            for i in range(DT):
                nc.sync.dma_start(t_wqkv[i][:], wqkvT[i * 128:(i + 1) * 128, :])
            for i in range(2):
                nc.sync.dma_start(t_wmod[i][:], wmodT[i * 128:(i + 1) * 128, :])

            def layernorm_bcast(x_tiles, cols):
                """LN stats over the feature (partition) axis for one
                512-token chunk. Returns (R, M) SBUF bf16 [128,512]
                broadcast tiles: xn = x*R - M."""
                ps_st = psum.tile([128, 512], F32, tag="gen", name="gen", bufs=2)
                xsq = [None] * DT
                for dt in range(DT):
                    xsq[dt] = work1.tile([128, 512], BF16, tag=f"xsq_{dt}", name=f"xsq_{dt}")
                    nc.scalar.square(xsq[dt][:], x_tiles[dt][:, cols])
                for dt in range(DT):
                    nc.tensor.matmul(ps_st[0:1, :], t_ones[:], x_tiles[dt][:, cols],
                                     start=(dt == 0), stop=(dt == DT - 1))
                for dt in range(DT):
                    nc.tensor.matmul(ps_st[64:65, :], t_ones[:], xsq[dt][:],
                                     start=(dt == 0), stop=(dt == DT - 1))
                mu = work1.tile([1, 512], F32, tag="mu", name="mu", bufs=2)
                msq = work1.tile([1, 512], F32, tag="msq", name="msq", bufs=2)
                nc.vector.tensor_scalar_mul(mu[:], ps_st[0:1, :], 1.0 / D)
                nc.vector.tensor_scalar_mul(msq[:], ps_st[64:65, :], 1.0 / D)
                mu2 = work1.tile([1, 512], F32, tag="mu2", name="mu2", bufs=2)
                nc.scalar.square(mu2[:], mu[:])
                var = work1.tile([1, 512], F32, tag="var", name="var", bufs=2)
                nc.vector.tensor_tensor(var[:], msq[:], mu2[:], ALU.subtract)
                lnv = work1.tile([1, 512], F32, tag="lnv", name="lnv", bufs=2)
                nc.scalar.activation(lnv[:], var[:], AF.Ln, bias=t_eps[:, 0:1])
                rstd = work1.tile([1, 512], BF16, tag="rstd", name="rstd", bufs=2)
                nc.scalar.activation(rstd[:], lnv[:], AF.Exp, scale=-0.5)
                mrs = work1.tile([1, 512], BF16, tag="mrs", name="mrs", bufs=2)
                nc.vector.tensor_tensor(mrs[:], mu[:], rstd[:], ALU.mult)
                R = work1.tile([128, 512], BF16, tag="Rbc", name="Rbc", bufs=2)
                M = work1.tile([128, 512], BF16, tag="Mbc", name="Mbc", bufs=2)
                nc.gpsimd.partition_broadcast(R[:], rstd[:])
                nc.gpsimd.partition_broadcast(M[:], mrs[:])
                return R, M

            # ============ Phase A body (one 512-slot kv chunk) ============
            def phaseA(c4, act_frac=2):
                cols = slice(c4 * 512, (c4 + 1) * 512)
                own = c4 >= 2
                xt = t_xo if own else t_xk
                xcols = slice((c4 % 2) * 512, (c4 % 2 + 1) * 512)
                R, M = layernorm_bcast(xt, xcols)
                silu = [work.tile([128, 512], BF16, tag="silu", name="silu")
                        for _ in range(2)]
                for i in range(2):
                    a_chunk = work.tile([128, 512], BF16, tag="achunk", name="achunk")
                    nc.sync.dma_start(a_chunk[:], aT[i * 128:(i + 1) * 128, cols])
                    # silu(x) = x / (1 + exp(-x)); exp keeps ACT in the
                    # natural_log_exp table set shared with attention/LN
                    en = work.tile([128, 512], BF16, tag="en", name="en", bufs=1)
                    nc.scalar.activation(en[:], a_chunk[:], AF.Exp, scale=-1.0)
                    ep = work.tile([128, 512], BF16, tag="ep", name="ep", bufs=1)
                    nc.vector.tensor_scalar_add(ep[:], en[:], 1.0)
                    er = work.tile([128, 512], BF16, tag="er", name="er", bufs=1)
                    with nc.allow_low_precision(reason="bf16 silu denom"):
                        nc.vector.reciprocal(er[:], ep[:])
                    nc.vector.tensor_tensor(silu[i][:], a_chunk[:], er[:], ALU.mult)

                jlist = range(24) if own else range(8)
                sh1 = [None] * DT
                sc1p = [None] * DT
                modsp = None
                for j in jlist:
                    jc = slice(j * 128, (j + 1) * 128)
                    ps = psum.tile([128, 512], F32, tag="gen", name="gen", bufs=2)
                    for i in range(2):
                        nc.tensor.matmul(ps[:], t_wmod[i][:, jc], silu[i][:],
                                         start=(i == 0), stop=(i == 1))
                    grp, dt_i = divmod(j, 4)
                    bias_col = t_bcols[:, 24 + j:25 + j]
                    if j < 4:
                        dst = work1.tile([128, 512], BF16, tag=f"sh1_{dt_i}", name=f"sh1_{dt_i}")
                        sh1[dt_i] = dst
                        dst_ap = dst[:]
                    elif j < 8:
                        dst = work1.tile([128, 512], BF16, tag=f"sc1_{dt_i}", name=f"sc1_{dt_i}")
                        sc1p[dt_i] = dst
                        dst_ap = dst[:]
                    else:
                        if dt_i == 0:
                            modsp = work.tile([128, DT, 512], BF16, tag="modsp", name="modsp", bufs=1)
                        dst_ap = modsp[:, dt_i, :]
                    if act_frac and j % act_frac == 0:
                        nc.scalar.activation(dst_ap, ps[:], AF.Identity,
                                             bias=bias_col)
                    else:
                        nc.vector.tensor_scalar_add(dst_ap, ps[:], bias_col)
                    if j >= 8 and dt_i == DT - 1:
                        nc.sync.dma_start(
                            mod_dram[grp - 2][:, :, (c4 - 2) * 512:(c4 - 1) * 512],
                            modsp[:])

                h1 = [None] * DT
                for dt in range(DT):
                    t1 = work1.tile([128, 512], BF16, tag="t1", name="t1")
                    nc.vector.tensor_tensor(t1[:], xt[dt][:, xcols], R[:], ALU.mult)
                    xn = work1.tile([128, 512], BF16, tag="xn", name="xn")
                    nc.vector.tensor_tensor(xn[:], t1[:], M[:], ALU.subtract)
                    t2 = work1.tile([128, 512], BF16, tag="t2", name="t2")
                    nc.vector.tensor_tensor(t2[:], xn[:], sc1p[dt][:], ALU.mult)
                    h1[dt] = work.tile([128, 512], BF16, tag=f"h1_{dt}", name=f"h1_{dt}")
                    nc.vector.tensor_tensor(h1[dt][:], t2[:], sh1[dt][:], ALU.add)

                # k projection (wqkv cols 512:1024)
                for j in range(4):
                    jc = slice(512 + j * 128, 512 + (j + 1) * 128)
                    ps = psum.tile([128, 512], F32, tag="gen", name="gen", bufs=2)
                    for dt in range(DT):
                        nc.tensor.matmul(ps[:], t_wqkv[dt][:, jc], h1[dt][:],
                                         start=(dt == 0), stop=(dt == DT - 1))
                    nc.vector.tensor_copy(t_k[j][:, cols], ps[:])
                # v projection (wqkv cols 1024:1536), token-major
                for tt in range(4):
                    trows = slice(tt * 128, (tt + 1) * 128)
                    ps = psum.tile([128, 512], F32, tag="gen", name="gen", bufs=2)
                    for dt in range(DT):
                        nc.tensor.matmul(ps[:], h1[dt][:, trows],
                                         t_wqkv[dt][:, 1024:1536],
                                         start=(dt == 0), stop=(dt == DT - 1))
                    vt = t_v[c4 * 4 + tt]
                    nc.vector.tensor_copy(
                        vt[:, :, 0:DH],
                        ps[:].rearrange("p (h d) -> p h d", d=DH))
                # q projection (own chunks only; wqkv cols 0:512)
                if own:
                    qcols = slice((c4 - 2) * 512, (c4 - 1) * 512)
                    for j in range(4):
                        jc = slice(j * 128, (j + 1) * 128)
                        ps = psum.tile([128, 512], F32, tag="gen", name="gen", bufs=2)
                        for dt in range(DT):
                            nc.tensor.matmul(ps[:], t_wqkv[dt][:, jc], h1[dt][:],
                                             start=(dt == 0), stop=(dt == DT - 1))
                        nc.vector.tensor_copy(t_q[j][:, qcols], ps[:])

            # ============ Phase B body (one 512-token q chunk) ============
            def phaseB(Q):
                ncv = 12 if Q == 0 else 16
                qcols = slice(Q * 512, (Q + 1) * 512)
                band = range(8, 12) if Q == 0 else range(12, 16)
                for hg in range(4):          # head pairs (2hg, 2hg+1)
                    ktile = t_k[hg]
                    qtile = t_q[hg]
                    ps_o = [psum.tile([DH + 1, 512], F32, tag="o2", name="o2", bufs=2)
                            for _ in range(2)]
                    for c in range(ncv):
                        kc = slice(c * 128, (c + 1) * 128)
                        ps_s = psum.tile([128, 1024], F32, tag="sc2g", name="sc2g", bufs=2)
                        for hi in range(2):
                            nc.tensor.matmul(ps_s[:, hi * 512:(hi + 1) * 512],
                                             ktile[hi * 64:hi * 64 + 64, kc],
                                             qtile[hi * 64:hi * 64 + 64, qcols],
                                             start=True, stop=True,
                                             tile_position=(hi * 64, 0))
                        E = work.tile([128, 2, 512], BF16, tag="E", name="E", bufs=2)
                        nc.scalar.activation(
                            E[:].rearrange("p a b -> p (a b)"),
                            ps_s[:], AF.Exp,
                            bias=t_kvb[:, c:c + 1], scale=ISQ)
                        if c in band:
                            theta = 1024 + 512 * Q - 128 * c
                            Em = work.tile([128, 2, 512], BF16, tag="Em", name="Em", bufs=2)
                            nc.gpsimd.affine_select(
                                Em[:], E[:], pattern=[[0, 2], [1, 512]],
                                compare_op=ALU.is_ge, fill=0.0,
                                base=theta, channel_multiplier=-1)
                            src = Em
                        else:
                            src = E
                        for hi in range(2):
                            nc.tensor.matmul(ps_o[hi][:],
                                             t_v[c][:, 2 * hg + hi, :],
                                             src[:, hi, :],
                                             start=(c == 0), stop=(c == ncv - 1))
                    for hi in range(2):
                        h = 2 * hg + hi
                        hrows = slice(hi * 64, hi * 64 + 64)
                        recip = work.tile([1, 512], BF16, tag="recip", name="recip")
                        with nc.allow_low_precision(reason="bf16 softmax denom"):
                            nc.vector.reciprocal(recip[:], ps_o[hi][DH:DH + 1, :])
                        rbc = work1.tile([64, 512], BF16, tag="rbc", name="rbc")
                        nc.gpsimd.partition_broadcast(rbc[:], recip[:])
                        nc.vector.tensor_tensor(t_o[hg][hrows, qcols],
                                                ps_o[hi][0:DH, :], rbc[:], ALU.mult)

            # ============ Phase C sub-phases ============
            def read_mod(g, T):
                rb = work1.tile([128, DT, 512], BF16, tag=f"rb{g}", name=f"rb{g}")
                nc.sync.dma_start(rb[:], mod_dram[g][:, :, T * 512:(T + 1) * 512])
                return rb

            def projC(T):
                tcols = slice(T * 512, (T + 1) * 512)
                xcols = slice(T * 512, (T + 1) * 512)
                gate1 = read_mod(0, T)
                for m in range(4):
                    mc = slice(m * 128, (m + 1) * 128)
                    ps = psum.tile([128, 512], F32, tag="gen", name="gen", bufs=2)
                    for it in range(DT):
                        nc.tensor.matmul(ps[:], t_wout[it][:, mc], t_o[it][:, tcols],
                                         start=(it == 0), stop=(it == DT - 1))
                    pb = work1.tile([128, 512], BF16, tag="pb", name="pb")
                    nc.vector.tensor_scalar_add(pb[:], ps[:], t_bcols[:, m:m + 1])
                    pg = work1.tile([128, 512], BF16, tag="pg", name="pg")
                    nc.vector.tensor_tensor(pg[:], pb[:], gate1[:, m, :], ALU.mult)
                    nc.vector.tensor_tensor(t_x1[m][:, tcols], pg[:],
                                            t_xo[m][:, xcols], ALU.add)

            def lnC(T):
                tcols = slice(T * 512, (T + 1) * 512)
                shift2 = read_mod(1, T)
                scale2p = read_mod(2, T)
                R, M = layernorm_bcast(t_x1, tcols)
                h2 = work.tile([128, DT, 512], FP8, tag="h2", name="h2", bufs=2)
                for dt in range(DT):
                    t1 = work1.tile([128, 512], BF16, tag="t1", name="t1")
                    nc.vector.tensor_tensor(t1[:], t_x1[dt][:, tcols], R[:], ALU.mult)
                    xn = work1.tile([128, 512], BF16, tag="xn", name="xn")
                    nc.vector.tensor_tensor(xn[:], t1[:], M[:], ALU.subtract)
                    t2 = work1.tile([128, 512], BF16, tag="t2", name="t2")
                    nc.vector.tensor_tensor(t2[:], xn[:], scale2p[:, dt, :], ALU.mult)
                    nc.vector.tensor_tensor(h2[:, dt, :], t2[:], shift2[:, dt, :], ALU.add)
                return h2

            def mlpC(T, h2, t_w1, t_w2):
                tcols = slice(T * 512, (T + 1) * 512)
                gate2 = read_mod(3, T)
                ps2pair = psum.tile([128, 1024], F32, tag="sc2g", name="sc2g", bufs=2)
                ps2 = [ps2pair[:, 0:512], ps2pair[:, 512:1024],
                       psum.tile([128, 512], F32, tag="o2", name="o2", bufs=2)[:],
                       psum.tile([128, 512], F32, tag="o2", name="o2", bufs=2)[:]]
                for kp8 in range(8):
                    mid = work.tile([128, 2, 512], FP8, tag="mid", name="mid", bufs=3)
                    for ki in range(2):
                        k16 = 2 * kp8 + ki
                        kc = slice(k16 * 128, (k16 + 1) * 128)
                        ps1 = psum.tile([128, 512], F32, tag="gen", name="gen", bufs=2)
                        for kp in range(2):
                            nc.tensor.matmul(
                                ps1[:], t_w1[:, 2 * kp:2 * kp + 2, kc],
                                h2[:, 2 * kp:2 * kp + 2, :],
                                start=(kp == 0), stop=(kp == 1), perf_mode=DR)
                        nc.scalar.activation(mid[:, ki, :], ps1[:], AF.Gelu,
                                             bias=t_bcols[:, 4 + k16:5 + k16])
                    for m in range(4):
                        mc = slice(m * 128, (m + 1) * 128)
                        nc.tensor.matmul(ps2[m], t_w2[:, 2 * kp8:2 * kp8 + 2, mc],
                                         mid[:], perf_mode=DR,
                                         start=(kp8 == 0), stop=(kp8 == 7))
                for m in range(4):
                    ob = work1.tile([128, 512], BF16, tag="pb", name="pb")
                    nc.vector.tensor_scalar_add(ob[:], ps2[m], t_bcols[:, 20 + m:21 + m])
                    og = work1.tile([128, 512], BF16, tag="pg", name="pg")
                    nc.vector.tensor_tensor(og[:], ob[:], gate2[:, m, :], ALU.mult)
                    ofin = work1.tile([128, 512], F32, tag="ofin", name="ofin")
                    nc.vector.tensor_tensor(ofin[:], og[:], t_x1[m][:, tcols], ALU.add)
                    nc.sync.dma_start(out[m * 128:(m + 1) * 128, tcols], ofin[:])

            # ---------------- program order ----------------
            phaseA(0)
            phaseA(1)
            phaseA(2)
            phaseB(0)
            phaseA(3, act_frac=0)
            poolWA.__exit__(None, None, None)
            poolWC = tc.tile_pool(name="wC", bufs=1)
            wC = poolWC.__enter__()
            t_w1 = wC.tile([128, DT, MLP], FP8, tag="w1dr", name="w1dr")
            t_w2 = wC.tile([128, 16, D], FP8, tag="w2dr", name="w2dr")
            nc.sync.dma_start(t_w1[:].rearrange("p a b -> p (a b)"), w1T[:])
            nc.sync.dma_start(t_w2[:].rearrange("p a b -> p (a b)"), w2T[:])
            phaseB(1)
            projC(0)
            projC(1)
            h2_0 = lnC(0)
            h2_1 = lnC(1)
            mlpC(0, h2_0, t_w1, t_w2)
            mlpC(1, h2_1, t_w1, t_w2)
            poolWC.__exit__(None, None, None)

    nc.compile()
    return nc


def _get_nc():
    if "nc" not in _cache:
        _cache["nc"] = _build()
    return _cache["nc"]


def _shard(x, action_emb, Wqkv, Wout, bout, W1, b1, W2, b2, Wmod, bmod):
    bf = ml_dtypes.bfloat16
    wqkvT = np.ascontiguousarray(Wqkv.T).astype(bf)
    wmodT = np.ascontiguousarray(Wmod.T).astype(bf)
    woutT = np.ascontiguousarray(Wout.T).astype(bf)
    f8 = ml_dtypes.float8_e4m3
    # DoubleRow layout: [partition, k_subtile, out_col]
    w1T = np.ascontiguousarray(
        W1.T.reshape(DT, 128, MLP).transpose(1, 0, 2).reshape(128, DT * MLP)
    ).astype(f8)
    w2T = np.ascontiguousarray(
        W2.T.reshape(16, 128, D).transpose(1, 0, 2).reshape(128, 16 * D)
    ).astype(f8)
    bcolsv = np.zeros((128, 48), np.float32)
    bcolsv[:, 0:4] = bout.reshape(4, 128).T
    bcolsv[:, 4:20] = b1.reshape(16, 128).T
    bcolsv[:, 20:24] = b2.reshape(4, 128).T
    bmod_cols = np.ascontiguousarray(bmod.reshape(24, 128).T)
    bmod_cols[:, 4:8] += 1.0    # scale1 -> 1 + scale1
    bmod_cols[:, 16:20] += 1.0  # scale2 -> 1 + scale2
    bcolsv[:, 24:48] = bmod_cols

    in_maps = []
    for b in range(4):
        for p in range(2):
            if p == 0:
                xw = np.zeros((TKV, D), np.float32)
                xw[1024:] = x[b, 0:1024]
                aw = np.zeros((TKV, ADIM), np.float32)
                aw[1024:] = action_emb[b, 0:1024]
                kvbv = np.zeros((128, 16), np.float32)
                kvbv[:, 0:8] = NEG
            else:
                xw = x[b]
                aw = action_emb[b]
                kvbv = np.zeros((128, 16), np.float32)
            in_maps.append({
                "xT": np.ascontiguousarray(xw.T).astype(bf),
                "aT": np.ascontiguousarray(aw.T).astype(bf),
                "kvb": kvbv,
                "wqkvT": wqkvT,
                "wmodT": wmodT,
                "woutT": woutT,
                "w1T": w1T,
                "w2T": w2T,
                "bcols": bcolsv,
            })
    return in_maps


def kernel(x, action_emb, causal_mask, Wqkv, Wout, bout, W1, b1, W2, b2,
           Wmod, bmod):
    in_maps = _shard(np.asarray(x, np.float32), np.asarray(action_emb, np.float32),
                     np.asarray(Wqkv, np.float32), np.asarray(Wout, np.float32),
                     np.asarray(bout, np.float32), np.asarray(W1, np.float32),
                     np.asarray(b1, np.float32), np.asarray(W2, np.float32),
                     np.asarray(b2, np.float32), np.asarray(Wmod, np.float32),
                     np.asarray(bmod, np.float32))
    nc = _get_nc()
    res = run_bass_kernel_spmd(nc, in_maps, core_ids=list(range(8)))
    full = np.empty((4, 2048, D), np.float32)
    for b in range(4):
        for p in range(2):
            full[b, p * 1024:(p + 1) * 1024, :] = res.results[2 * b + p]["out"].T
    return full
